# revision 1
# baseline (speedup 1.0000x reference)
"""Trainium2 Bass kernel for MDMLPPatch (3x3 unfold + per-channel linear 9->64).

out[n,c,p,e] = sum_d patches[n,c,p,d] * W[d,e] + b[e]
x: [16,64,56,56] f32, W: [9,64] f32, b: [64] f32 -> out: [16,64,3136,64] f32

Sharding: data-parallel over batch N: 16 n / 8 cores = 2 n per core.
Each core processes 128 independent 56x56 images (2 n x 64 c).

Layout (per image, 3136 pixels):
  - 12 "pair tiles" of 256 pixels + 64 tail pixels.
  - u-order: pixel p (p < 3072): T = p//256, par = p%2, idx = (p%256)//2,
    u = 256*T + 128*par + idx. Tail (q = p-3072): u = 3072 + 32*(q%2) + q//2.
  - The host ships S[img, d, u] = patches in u-order (d=0..8 are the 9 taps,
    d=9 is all-ones so the K=10 matmul contraction adds the bias for free).

Per-core kernel, per image:
  - one contiguous DMA loads S[img] -> SBUF sh[10, 3136]
  - 26 matmuls, all with contiguous stride-1 lhsT slices (even/odd pixel
    halves of each pair tile): lhsT = sh[:, 128k:128k+128], rhs = W' [10,64]
    -> PSUM partition q of a pair tile holds pixels (256T+2q, 256T+2q+1)
    side by side = 512B contiguous DRAM runs; no transpose anywhere.
  - 8 MM outputs fill one PSUM bank [128, 512]; DVE/ACT copy each bank into
    a large SBUF staging buffer
  - per 8-image group: 9 DMAs out, all 512B descriptors, contiguous DRAM.
"""

import numpy as np

import concourse.bass as bass
import concourse.mybir as mybir
from concourse import bacc
from concourse.tile import TileContext
from concourse.bass_utils import run_bass_kernel_spmd

F32 = mybir.dt.float32

N_CORES = 8
IMGS = 128            # images per core (2 n x 64 c)
NPIX = 56 * 56        # 3136
KDIM = 10             # 9 taps + ones (bias) row
PAIR_TILES = 12       # 256-pixel tiles per image
TAIL_PIX = 64
GROUP_IMGS = 8
IMG_COLS = PAIR_TILES * 128       # 1536 stage cols per image (full tiles)
STAGE_COLS = GROUP_IMGS * IMG_COLS + GROUP_IMGS * 128  # + tail region


def build_nc(imgs=IMGS, group_imgs=GROUP_IMGS, psum_bufs=5, n_sh=4,
             do_mm=True, do_copy=True, do_out=True, repeat=1):
    n_groups = imgs // group_imgs
    stage_cols = group_imgs * IMG_COLS + group_imgs * 128
    assert group_imgs % 4 == 0

    nc = bacc.Bacc("TRN2", target_bir_lowering=False, debug=False)
    sd = nc.dram_tensor("s", [imgs, KDIM, NPIX], F32, kind="ExternalInput")
    wd = nc.dram_tensor("w", [KDIM, 64], F32, kind="ExternalInput")
    out = nc.dram_tensor("out", [imgs * NPIX * 64], F32, kind="ExternalOutput")

    with TileContext(nc) as tc:
        with (
            tc.tile_pool(name="const", bufs=1) as constp,
            tc.tile_pool(name="shift", bufs=n_sh) as shiftp,
            tc.tile_pool(name="stage", bufs=2) as stagep,
            tc.tile_pool(name="psum", bufs=psum_bufs, space="PSUM") as psump,
            tc.tile_pool(name="psumt", bufs=2, space="PSUM") as psumt,
        ):
            wt = constp.tile([KDIM, 64], F32)
            nc.sync.dma_start(out=wt[:, :], in_=wd[:, :])
            if not do_out:
                dummy = bass.AP(out, 0, [[64, KDIM], [1, 64]])
                nc.sync.dma_start(out=dummy, in_=wt[:, :])

            copy_idx = 0
            for g_iter in range(n_groups * repeat):
                g = g_iter % n_groups
                stage = stagep.tile([128, stage_cols], F32, tag="stage")
                tail_base = group_imgs * IMG_COLS
                ptail = None
                for li in range(group_imgs):
                    img = g * group_imgs + li
                    sh = shiftp.tile([KDIM, NPIX], F32, tag="sh")
                    # one contiguous load per image; alternate DGE paths so
                    # loads never queue behind the big out-DMAs (SP HWDGE)
                    eng = nc.scalar if img % 2 == 0 else nc.gpsimd
                    eng.dma_start(out=sh[:, :], in_=sd[img])
                    if li % 4 == 0 and do_mm:
                        ptail = psumt.tile([128, 512], F32, tag="ptail")
                    # 24 full MMs -> 3 banks of 4 pair-tiles; lhsT slices are
                    # contiguous u-blocks (even/odd pixel halves).
                    for bank in range(3):
                        if not do_mm:
                            break
                        pfull = psump.tile([128, 512], F32, tag="pfull")
                        for s in range(4):
                            T = 4 * bank + s
                            for par in range(2):
                                k = 2 * T + par
                                lhsT = sh[0:KDIM, 128 * k:128 * (k + 1)]
                                nc.tensor.matmul(
                                    out=pfull[:, 128 * s + 64 * par:
                                              128 * s + 64 * par + 64],
                                    lhsT=lhsT, rhs=wt[:, :],
                                    start=True, stop=True,
                                )
                        if do_copy:
                            dst = stage[:, li * IMG_COLS + 512 * bank:
                                        li * IMG_COLS + 512 * (bank + 1)]
                            if copy_idx % 2 == 0:
                                nc.vector.tensor_copy(dst, pfull[:, :])
                            else:
                                nc.scalar.copy(dst, pfull[:, :])
                            copy_idx += 1
                    # tail: 64 leftover pixels -> 2 MMs of M=32 into the
                    # shared per-4-image tail bank at col block 128*(li%4)
                    for par in range(2):
                        if not do_mm:
                            break
                        lhsT = sh[0:KDIM, 3072 + 32 * par:3072 + 32 * (par + 1)]
                        nc.tensor.matmul(
                            out=ptail[0:32, 128 * (li % 4) + 64 * par:
                                      128 * (li % 4) + 64 * par + 64],
                            lhsT=lhsT, rhs=wt[:, :], start=True, stop=True,
                        )
                    if li % 4 == 3 and do_mm and do_copy:
                        dst = stage[0:32, tail_base + 512 * (li // 4):
                                    tail_base + 512 * (li // 4 + 1)]
                        if copy_idx % 2 == 0:
                            nc.vector.tensor_copy(dst, ptail[0:32, :])
                        else:
                            nc.scalar.copy(dst, ptail[0:32, :])
                        copy_idx += 1
                # ---- group DMAs out (all 512B descriptors) ----
                # src APs are tile-derived so Tile tracks RAW/WAR deps on
                # `stage`; the DRAM side (write-only, never read) is raw.
                base = g * group_imgs * NPIX * 64
                if not do_out:
                    continue
                for li in range(group_imgs):
                    out_full = bass.AP(
                        out, base + li * NPIX * 64,
                        [[128, 128], [256 * 64, PAIR_TILES], [1, 128]],
                    )
                    src_full = stage[:, li * IMG_COLS:(li + 1) * IMG_COLS]
                    nc.sync.dma_start(out=out_full, in_=src_full)
                out_tail = bass.AP(
                    out, base + (NPIX - TAIL_PIX) * 64,
                    [[128, 32], [NPIX * 64, group_imgs], [1, 128]],
                )
                src_tail = stage[0:32, tail_base:tail_base + group_imgs * 128]
                nc.sync.dma_start(out=out_tail, in_=src_tail)
    nc.compile()
    return nc


_CACHE = {}


def _get_nc(imgs=IMGS, group_imgs=GROUP_IMGS):
    key = (imgs, group_imgs)
    if key not in _CACHE:
        _CACHE[key] = build_nc(imgs, group_imgs)
    return _CACHE[key]


def _u_perm():
    """p_of_u[u] = pixel index stored at u-position u."""
    p = np.arange(NPIX - TAIL_PIX)
    T, r = np.divmod(p, 256)
    par, idx = r % 2, r // 2
    u_full = 256 * T + 128 * par + idx
    q = np.arange(TAIL_PIX)
    u_tail = (NPIX - TAIL_PIX) + 32 * (q % 2) + q // 2
    u_of_p = np.concatenate([u_full, u_tail])
    p_of_u = np.empty(NPIX, dtype=np.int64)
    p_of_u[u_of_p] = np.arange(NPIX)
    return p_of_u


_P_OF_U = _u_perm()


def _prep_inputs(x, W, b):
    x = np.ascontiguousarray(np.asarray(x, dtype=np.float32))
    W = np.ascontiguousarray(np.asarray(W, dtype=np.float32))
    b = np.ascontiguousarray(np.asarray(b, dtype=np.float32))
    N, C, H, Wd = x.shape
    nimg = N * C
    xpad = np.zeros((nimg, 58, 58), dtype=np.float32)
    xpad[:, 1:57, 1:57] = x.reshape(nimg, H, Wd)
    # S[img, d, p] = xpad[img, p//56 + d//3, p%56 + d%3]; d=9 -> ones
    S = np.empty((nimg, KDIM, NPIX), dtype=np.float32)
    for d in range(9):
        di, dj = divmod(d, 3)
        S[:, d, :] = xpad[:, di:di + 56, dj:dj + 56].reshape(nimg, NPIX)
    S[:, 9, :] = 1.0
    S = S[:, :, _P_OF_U]                      # u-order
    S = np.ascontiguousarray(S.reshape(N_CORES, nimg // N_CORES, KDIM, NPIX))
    wb = np.concatenate([W, b[None, :]], axis=0).astype(np.float32)  # [10,64]
    in_maps = [{"s": S[i], "w": wb} for i in range(N_CORES)]
    return in_maps, N, C


def run(x, W, b, trace=False, **kw):
    in_maps, N, C = _prep_inputs(x, W, b)
    nc = _get_nc()
    res = run_bass_kernel_spmd(
        nc, in_maps, core_ids=list(range(N_CORES)), trace=trace, **kw
    )
    outs = [
        res.results[i]["out"].reshape(N // N_CORES, C, NPIX, 64)
        for i in range(N_CORES)
    ]
    full = np.concatenate(outs, axis=0)
    return full, res


def kernel(x, W, b):
    full, _ = run(x, W, b, trace=False)
    return full


# ---------------------------------------------------------------------------
# benchmarking helpers (not used by the grading harness)
# ---------------------------------------------------------------------------

def bench(x, W, b, iters=20, warmup=3):
    """Wall-clock the NEFF execution via PJRT with device-resident inputs.

    Outputs of iteration i are donated as the (fully overwritten) output
    buffers of iteration i+1, so no zero-init cost is on the timed path.
    """
    import time
    import jax
    from jax.sharding import Mesh, PartitionSpec, NamedSharding
    from jax.experimental.shard_map import shard_map
    from concourse import bass2jax as b2j

    b2j.install_neuronx_cc_hook()
    in_maps, N, C = _prep_inputs(x, W, b)
    nc = _get_nc()

    partition_name = (
        nc.partition_id_tensor.name if nc.partition_id_tensor else None
    )
    in_names, out_names, out_avals = [], [], []
    for alloc in nc.m.functions[0].allocations:
        if not isinstance(alloc, mybir.MemoryLocationSet):
            continue
        name = alloc.memorylocations[0].name
        if alloc.kind == "ExternalInput":
            if name != partition_name:
                in_names.append(name)
        elif alloc.kind == "ExternalOutput":
            out_names.append(name)
            shape = tuple(alloc.tensor_shape)
            dtype = mybir.dt.np(alloc.dtype)
            out_avals.append(jax.core.ShapedArray(shape, dtype))
    n_params = len(in_names)
    n_outs = len(out_avals)
    all_names = in_names + out_names
    if partition_name is not None:
        all_names = all_names + [partition_name]

    def _body(*args):
        operands = list(args)
        if partition_name is not None:
            operands.append(b2j.partition_id_tensor())
        outs = b2j._bass_exec_p.bind(
            *operands,
            out_avals=tuple(out_avals),
            in_names=tuple(all_names),
            out_names=tuple(out_names),
            lowering_input_output_aliases=(),
            sim_require_finite=True,
            sim_require_nnan=True,
            nc=nc,
        )
        return tuple(outs)

    devices = jax.devices()[:N_CORES]
    mesh = Mesh(np.asarray(devices), ("core",))
    donate = tuple(range(n_params, n_params + n_outs))
    fn = jax.jit(
        shard_map(
            _body, mesh=mesh,
            in_specs=(PartitionSpec("core"),) * (n_params + n_outs),
            out_specs=(PartitionSpec("core"),) * n_outs,
            check_rep=False,
        ),
        donate_argnums=donate, keep_unused=True,
    )
    concat_in = [
        np.concatenate([np.asarray(m[nm]) for m in in_maps], axis=0)
        for nm in in_names
    ]
    sh = NamedSharding(mesh, PartitionSpec("core"))
    dev_in = [jax.device_put(a, sh) for a in concat_in]
    outs = tuple(
        jax.device_put(
            np.zeros((N_CORES * a.shape[0], *a.shape[1:]), a.dtype), sh
        )
        for a in out_avals
    )
    times = []
    for i in range(warmup + iters):
        t0 = time.perf_counter()
        outs = fn(*dev_in, *outs)
        jax.block_until_ready(outs)
        t1 = time.perf_counter()
        if i >= warmup:
            times.append(t1 - t0)
    t0 = time.perf_counter()
    for _ in range(iters):
        outs = fn(*dev_in, *outs)
    jax.block_until_ready(outs)
    piped = (time.perf_counter() - t0) / iters
    out_np = [np.asarray(o) for o in outs]
    return times, {"piped": piped, **dict(zip(out_names, out_np))}


def timeline(out_path=None, imgs=16, group_imgs=GROUP_IMGS):
    """Cost-model simulation of a reduced-size variant; returns modeled ns."""
    from concourse.timeline_sim import TimelineSim
    nc = build_nc(imgs=imgs, group_imgs=group_imgs)
    ts = TimelineSim(nc, trace=False)
    return ts.simulate()



# revision 3
# speedup vs baseline: 21.0474x; 21.0474x over previous
"""Trainium2 Bass kernel v6 for MDMLPPatch (3x3 unfold + per-channel linear 9->64).

out[n,c,p,e] = sum_d patches[n,c,p,d] * W[d,e] + b[e]
x: [16,64,56,56] f32, W: [9,64] f32, b: [64] f32 -> out: [16,64,3136,64] f32

Data-parallel over batch N: 16 n / 8 cores = 2 n per core -> 128 images/core.

Block-diagonal-weight scheme, full-width M=128 matmuls + batched tails:
  - K = 80 = 10 taps x 8 pixel-phases (taps include a ones-row for bias).
    Host ships, per image, S2[8d+j, col] = patches[d, 8*col + j] in bf16
    ([80, 392]); and w2[8d+j', 64j+e] = Wb[d,e]*(j==j') in bf16 ([80, 512]).
  - Image body = 3 chunks of 1024 pixels: one matmul per chunk
    (lhsT = S2[:, 128b:128b+128], rhs = w2) fills PSUM [128, 512] where
    partition q = pixels (1024b+8q .. +7) x 64ch = one contiguous DRAM run.
  - The 64-px tails of 16 images batch into ONE matmul [128, 512] via a
    3-dim lhsT AP (partition P = 8i+q <-> image i tail partition q); its
    out-DMA uses a 2-level partition decomposition [[200704,16],[512,8]].
  - Per image: 3 matmuls, one [128, 1536] DVE/ACT cast-copy (f32->bf16),
    one out-DMA.  Per 16-image block: 1 load, 1 tail matmul/copy/DMA.
  - Output is written bf16 (rel-err ~4e-3 << 2e-2 gate); host casts to f32.
"""

import numpy as np
import ml_dtypes

import concourse.bass as bass
import concourse.mybir as mybir
from concourse import bacc
from concourse.tile import TileContext
from concourse.bass_utils import run_bass_kernel_spmd

F32 = mybir.dt.float32
BF16 = mybir.dt.bfloat16
NP_BF16 = ml_dtypes.bfloat16

N_CORES = 8
IMGS = 128            # images per core (2 n x 64 c)
NPIX = 56 * 56        # 3136
KDIM = 10             # 9 taps + ones (bias) row
G = 8                 # pixels per partition-run
NCOL = NPIX // G      # 392
K2 = KDIM * G         # 80
N2 = G * 64           # 512
IMG64 = NPIX * 64     # 200704 elements per image


def build_nc(imgs=IMGS, blk=16, psum_bufs=2, sh_bufs=2, stage_bufs=4,
             do_mm=True, do_copy=True, do_out=True, repeat=1,
             in_eng="scalar", out_eng="sync", out_bf16=True, pool_copy=0):
    assert imgs % blk == 0
    ODT = BF16 if out_bf16 else F32
    nc = bacc.Bacc("TRN2", target_bir_lowering=False, debug=False)
    sd = nc.dram_tensor("s", [K2, imgs, NCOL], BF16, kind="ExternalInput")
    wd = nc.dram_tensor("w", [K2, N2], BF16, kind="ExternalInput")
    out = nc.dram_tensor("out", [imgs * IMG64], ODT, kind="ExternalOutput")

    with TileContext(nc) as tc:
        with (
            tc.tile_pool(name="const", bufs=1) as constp,
            tc.tile_pool(name="shift", bufs=sh_bufs) as shiftp,
            tc.tile_pool(name="stage", bufs=stage_bufs) as stagep,
            tc.tile_pool(name="tstage", bufs=2) as tstagep,
            tc.tile_pool(name="tgat", bufs=2) as tgatp,
            tc.tile_pool(name="psum", bufs=psum_bufs, space="PSUM") as psump,
            tc.tile_pool(name="ptail", bufs=2, space="PSUM") as ptailp,
        ):
            wt = constp.tile([K2, N2], BF16)
            nc.sync.dma_start(out=wt[:, :], in_=wd[:, :])
            if not do_out:
                dummyt = constp.tile([K2, N2], ODT)
                nc.vector.tensor_copy(dummyt[:, :], wt[:, :])
                dummy = bass.AP(out, 0, [[N2, K2], [1, N2]])
                nc.sync.dma_start(out=dummy, in_=dummyt[:, :])

            in_dma = getattr(nc, in_eng).dma_start
            out_dma = getattr(nc, out_eng).dma_start
            copy_idx = 0

            def do_one_copy(dst, src):
                nonlocal copy_idx
                if pool_copy and copy_idx % pool_copy == pool_copy - 1:
                    nc.gpsimd.tensor_copy(dst, src)
                elif copy_idx % 2 == 0:
                    nc.vector.tensor_copy(dst, src)
                else:
                    nc.scalar.copy(dst, src)
                copy_idx += 1

            sh = None
            for it in range(imgs * repeat):
                img = it % imgs
                ib = img % blk
                if ib == 0:
                    sh = shiftp.tile([K2, blk * NCOL], BF16, tag="sh")
                    in_dma(out=sh[:, :], in_=sd[:, img:img + blk, :])
                    if do_mm:
                        # one batched tail matmul for the whole block:
                        # gather 16 images' tail cols -> contiguous [80, 128]
                        # (walrus rejects multi-free-dim matmul weights, so
                        # go through a tiny DVE gather-copy first);
                        # lhsT col (i, q) -> PSUM partition 8i+q
                        shap = sh[:, :]
                        src = bass.AP(
                            shap.tensor, shap.offset + (NCOL - G),
                            [list(shap.ap[0]), [NCOL, blk], [1, G]],
                        )
                        tg = tgatp.tile([K2, blk * G], BF16, tag="tg")
                        nc.vector.tensor_copy(tg[:, :], src)
                        pt = ptailp.tile([128, N2], F32, tag="pt")
                        nc.tensor.matmul(out=pt[:, :], lhsT=tg[:, :],
                                         rhs=wt[:, :], start=True, stop=True)
                        if do_copy:
                            ts = tstagep.tile([128, N2], ODT, tag="ts")
                            do_one_copy(ts[:, :], pt[:, :])
                            if do_out:
                                tail_ap = bass.AP(
                                    out, img * IMG64 + (NPIX - 64) * 64,
                                    [[IMG64, blk], [N2, G], [1, N2]],
                                )
                                out_dma(out=tail_ap, in_=ts[:, :])
                stage = stagep.tile([128, 3 * N2], ODT, tag="stage")
                if do_mm:
                    p = psump.tile([128, 3 * N2], F32, tag="p")
                    for b in range(3):
                        nc.tensor.matmul(
                            out=p[:, N2 * b:N2 * (b + 1)],
                            lhsT=sh[:, ib * NCOL + 128 * b:
                                    ib * NCOL + 128 * (b + 1)],
                            rhs=wt[:, :], start=True, stop=True,
                        )
                    if do_copy:
                        do_one_copy(stage[:, :], p[:, :])
                if not do_out:
                    continue
                out_ap = bass.AP(
                    out, img * IMG64,
                    [[N2, 128], [1024 * 64, 3], [1, N2]],
                )
                out_dma(out=out_ap, in_=stage[:, :])
    nc.compile()
    return nc


_CACHE = {}


def _get_nc(**kw):
    key = tuple(sorted(kw.items()))
    if key not in _CACHE:
        _CACHE[key] = build_nc(**kw)
    return _CACHE[key]


def _prep_inputs(x, W, b):
    x = np.ascontiguousarray(np.asarray(x, dtype=np.float32))
    W = np.ascontiguousarray(np.asarray(W, dtype=np.float32))
    b = np.ascontiguousarray(np.asarray(b, dtype=np.float32))
    N, C, H, Wd = x.shape
    nimg = N * C
    xpad = np.zeros((nimg, 58, 58), dtype=np.float32)
    xpad[:, 1:57, 1:57] = x.reshape(nimg, H, Wd)
    # S[img, d, p] = xpad[img, p//56 + d//3, p%56 + d%3]; d=9 -> ones
    S = np.empty((nimg, KDIM, NPIX), dtype=np.float32)
    for d in range(9):
        di, dj = divmod(d, 3)
        S[:, d, :] = xpad[:, di:di + 56, dj:dj + 56].reshape(nimg, NPIX)
    S[:, 9, :] = 1.0
    # S2[img, 8d+j, col] = S[img, d, 8*col + j]
    S2 = S.reshape(nimg, KDIM, NCOL, G).transpose(0, 1, 3, 2)
    S2 = S2.reshape(nimg, K2, NCOL)
    # per-core [K2, IMGS, NCOL] so 16-image loads are 12.5 KB/descriptor
    S2T = np.ascontiguousarray(
        S2.reshape(N_CORES, IMGS, K2, NCOL).transpose(0, 2, 1, 3)
    ).astype(NP_BF16)
    Wb = np.concatenate([W, b[None, :]], axis=0).astype(np.float32)  # [10,64]
    w2 = np.zeros((KDIM, G, G, 64), dtype=np.float32)
    for j in range(G):
        w2[:, j, j, :] = Wb
    w2 = np.ascontiguousarray(w2.reshape(K2, N2)).astype(NP_BF16)
    in_maps = [{"s": S2T[i], "w": w2} for i in range(N_CORES)]
    return in_maps, N, C


def run(x, W, b, trace=False, **kw):
    in_maps, N, C = _prep_inputs(x, W, b)
    nc = _get_nc()
    res = run_bass_kernel_spmd(
        nc, in_maps, core_ids=list(range(N_CORES)), trace=trace, **kw
    )
    outs = [
        res.results[i]["out"].astype(np.float32).reshape(
            N // N_CORES, C, NPIX, 64)
        for i in range(N_CORES)
    ]
    full = np.concatenate(outs, axis=0)
    return full, res


def kernel(x, W, b):
    full, _ = run(x, W, b, trace=False)
    return full
